# revision 1
# baseline (speedup 1.0000x reference)
"""Trainium2 Bass kernel for a 6-layer causal decoder transformer.

Model: B=128, T=256, E=384, H=6, D=64, DFF=1536, L=6, V=65 (f32 reference).
Sharding: pure data-parallel over batch across 8 NeuronCores (16 batches
per core), parameters replicated, no collectives.

Per-core device strategy:
  - Residual stream x kept SBUF-resident, token-major [128 tok, 384] f32.
  - All matmul operands bf16 (PE 1 cyc/row vs 4 for f32); f32 PSUM accum;
    f32 softmax statistics and residual stream for accuracy.
  - LayerNorm affine (g, b) folded into the following weight matrices on the
    host; attention scale 1/sqrt(D) folded into Wq. Device LN = bn_stats +
    Newton-iteration rsqrt on VectorE (keeps Sqrt off ScalarE so the one
    activation-table set {exp, identity, copy, relu} is never reloaded).
  - Embedding lookup as a one-hot matmul (one-hot built host-side).
  - Attention per (batch, head): feature-major q,k -> scores on PE; causal
    mask added by PE (identity @ mask accumulated into the scores PSUM);
    exp+row-sum fused in one ScalarE activation (max-shift elided: scores
    are O(1) for this model); P normalized on DVE, transposed via PE
    (is_transpose matmul); attn@v with token-major v; per head-pair AV
    results accumulate into one PSUM bank -> single [128,256] copy-out.
  - Token-major/feature-major transposes on PE + DVE/ScalarE copy-outs
    (measured faster than DMA-xbar transposes on this fabric).
  - Bias adds elided when all bias inputs are zero (true for this
    problem's setup_inputs); non-zero biases fall back to a full-bias
    program variant.
"""

import sys
from contextlib import ExitStack

sys.path.insert(0, "/opt/trn_rl_repo")

import numpy as np
import ml_dtypes

import concourse.bass as bass
import concourse.bacc as bacc
import concourse.mybir as mybir
import concourse.tile as tile
from concourse.masks import make_identity
from concourse.bass_utils import run_bass_kernel_spmd

F32 = mybir.dt.float32
BF16 = mybir.dt.bfloat16
AF = mybir.ActivationFunctionType
OP = mybir.AluOpType
AX = mybir.AxisListType

P = 128
E, DFF, H, D, T, L, V = 384, 1536, 6, 64, 256, 6, 65
B = 128
N_CORES = 8
B_LOC = B // N_CORES          # 16 batches per core
NTOK = B_LOC * T              # 4096 tokens per core
NT = NTOK // P                # 32 token tiles
GROUP = 512                   # tokens per group (2 full batches)
NG = NTOK // GROUP            # 8 groups
TPG = GROUP // P              # 4 token tiles per group
BPG = GROUP // T              # 2 batches per group
EC = E // P                   # 3 feature chunks
FC = DFF // P                 # 12 dff chunks
NEG = -1.0e9

_PROG = None  # (nc, input_names)


def _ln_stats_group(nc, stat, x_list, eps=1e-5):
    """bn_stats per tile + batched Newton rsqrt. Returns (mv_g, rs_g):
    mv_g[:, i, 0:1] = mean of tile i; rs_g[:, i:i+1] = rsqrt(var_i + eps)."""
    n = len(x_list)
    mv_g = stat.tile([P, n, 2], F32, tag="mvg")
    for i, xin in enumerate(x_list):
        st6 = stat.tile([P, 6], F32, tag="bn6")
        nc.vector.bn_stats(out=st6[:], in_=xin)
        nc.vector.bn_aggr(out=mv_g[:, i, :], in_=st6[:])
    var = stat.tile([P, n], F32, tag="vare")
    nc.vector.tensor_scalar_add(var[:], mv_g[:, :, 1], eps)
    u = stat.tile([P, n], F32, tag="ue")
    nc.vector.reciprocal(u[:], var[:])
    lin = stat.tile([P, n], F32, tag="line")
    nc.vector.tensor_scalar(lin[:], var[:], 0.73, 0.32, op0=OP.mult, op1=OP.add)
    rs = stat.tile([P, n], F32, tag="rse")
    nc.vector.tensor_tensor(rs[:], u[:], lin[:], OP.mult)       # seed ~ rsqrt
    t1 = stat.tile([P, n], F32, tag="t1e")
    for _ in range(2):                                          # Newton x2
        nc.vector.tensor_tensor(t1[:], rs[:], rs[:], OP.mult)
        nc.vector.tensor_tensor(t1[:], t1[:], var[:], OP.mult)
        nc.vector.tensor_scalar(t1[:], t1[:], -0.5, 1.5, op0=OP.mult, op1=OP.add)
        nc.vector.tensor_tensor(rs[:], rs[:], t1[:], OP.mult)
    return mv_g, rs


def _ln_apply(nc, xin, xhat_out, mv_g, rs, i):
    nc.vector.tensor_scalar(xhat_out, xin, mv_g[:, i, 0:1], rs[:, i : i + 1],
                            op0=OP.subtract, op1=OP.mult)


def _copy_out(nc, dst, src, ctr):
    """PSUM->SBUF copy, alternating DVE / ScalarE."""
    if ctr[0] % 2 == 0:
        nc.vector.tensor_copy(dst, src)
    else:
        nc.scalar.copy(dst, src)
    ctr[0] += 1


def build_program(repeat=1, dma_t=False, zero_bias=True, pool_norm=False):
    nc = bacc.Bacc("TRN2", target_bir_lowering=False, debug=False,
                   num_devices=N_CORES)

    # register const APs needed for float biases on ScalarE activations
    for val in (1e-5,):
        t = nc.alloc_sbuf_tensor(f"const-f32-{val}", [P, 1], F32)
        nc.gpsimd.memset(t.ap(), val)
        nc.const_aps.aps[(F32, val)] = t.ap()
    nc.all_engine_barrier()

    # ---- I/O -------------------------------------------------------------
    oht = nc.dram_tensor("oht", [P, NTOK], BF16, kind="ExternalInput").ap()
    embp = nc.dram_tensor("embp", [P, E], BF16, kind="ExternalInput").ap()
    pose = nc.dram_tensor("pose", [T, E], F32, kind="ExternalInput").ap()
    maskd = nc.dram_tensor("maskd", [P, P], BF16, kind="ExternalInput").ap()
    wq = nc.dram_tensor("wq", [L, E, E], BF16, kind="ExternalInput").ap()
    wk = nc.dram_tensor("wk", [L, E, E], BF16, kind="ExternalInput").ap()
    wv = nc.dram_tensor("wv", [L, E, E], BF16, kind="ExternalInput").ap()
    wo = nc.dram_tensor("wo", [L, E, E], BF16, kind="ExternalInput").ap()
    w1 = nc.dram_tensor("w1", [L, E, DFF], BF16, kind="ExternalInput").ap()
    w2 = nc.dram_tensor("w2", [L, DFF, E], BF16, kind="ExternalInput").ap()
    wl = nc.dram_tensor("wl", [E, V], BF16, kind="ExternalInput").ap()
    bqf = nc.dram_tensor("bqf", [L, P, EC], F32, kind="ExternalInput").ap()
    bkf = nc.dram_tensor("bkf", [L, P, EC], F32, kind="ExternalInput").ap()
    c1f = nc.dram_tensor("c1f", [L, P, FC], F32, kind="ExternalInput").ap()
    btm = nc.dram_tensor("btm", [L, 3, P, E], F32, kind="ExternalInput").ap()
    blr = nc.dram_tensor("blr", [P, V], F32, kind="ExternalInput").ap()
    out = nc.dram_tensor("out", [NTOK, V], F32, kind="ExternalOutput").ap()

    with tile.TileContext(nc) as tc, ExitStack() as es:
            ep = es.enter_context
            const = ep(tc.tile_pool(name="const", bufs=1))
            xres = ep(tc.tile_pool(name="xres", bufs=1))
            wa = ep(tc.tile_pool(name="wa", bufs=2))
            wf = ep(tc.tile_pool(name="wf", bufs=2))
            bias = ep(tc.tile_pool(name="bias", bufs=2))
            grp = ep(tc.tile_pool(name="grp", bufs=2))
            grp1 = ep(tc.tile_pool(name="grp1", bufs=1))
            vt = ep(tc.tile_pool(name="vt", bufs=6))
            tk = ep(tc.tile_pool(name="tk", bufs=4))
            bh = ep(tc.tile_pool(name="bh", bufs=4))
            stat = ep(tc.tile_pool(name="stat", bufs=8))
            psmm = ep(tc.tile_pool(name="psmm", bufs=3, space="PSUM"))
            pstr = ep(tc.tile_pool(name="pstr", bufs=2, space="PSUM"))
            pss = ep(tc.tile_pool(name="pss", bufs=2, space="PSUM"))
            psav = ep(tc.tile_pool(name="psav", bufs=1, space="PSUM"))
            # ---- constants ----
            id_bf = const.tile([P, P], BF16, tag="id_bf")
            make_identity(nc, id_bf)
            mask_sb = const.tile([P, P], BF16, tag="mask")
            nc.sync.dma_start(mask_sb[:], maskd[:])
            emb_sb = const.tile([P, E], BF16, tag="emb")
            nc.sync.dma_start(emb_sb[:], embp[:])
            pose_sb = const.tile([P, 2, E], F32, tag="pose")
            nc.sync.dma_start(pose_sb[:, 0, :], pose[0:P, :])
            nc.sync.dma_start(pose_sb[:, 1, :], pose[P : 2 * P, :])
            wl_sb = const.tile([P, EC, V], BF16, tag="wl")
            nc.sync.dma_start(wl_sb[:], wl.rearrange("(kc p) n -> p kc n", p=P))
            blr_sb = const.tile([P, V], F32, tag="blr")
            nc.sync.dma_start(blr_sb[:], blr[:])
            oht_sb = const.tile([P, NTOK], BF16, tag="oht")
            nc.sync.dma_start(oht_sb[:], oht[:])

            _tctr = [0]
            x_tm = [xres.tile([P, E], F32, tag=f"x{t}", name=f"x{t}") for t in range(NT)]
            for _rep in range(repeat):
                # ---- x0 = onehot @ emb + pos ----
                for tt in range(NT):
                    xt = x_tm[tt]
                    pe = psmm.tile([P, GROUP], F32, tag="mm")
                    nc.tensor.matmul(pe[:, :E], oht_sb[:, tt * P : (tt + 1) * P],
                                     emb_sb[:], start=True, stop=True)
                    nc.vector.tensor_tensor(xt[:], pe[:, :E], pose_sb[:, tt % 2, :], OP.add)

                # ---- layers ----
                for l in range(L):
                    wq_sb = wa.tile([P, EC, E], BF16, tag="wq")
                    nc.sync.dma_start(wq_sb[:], wq[l].rearrange("(kc p) n -> p kc n", p=P))
                    wk_sb = wa.tile([P, EC, E], BF16, tag="wk")
                    nc.sync.dma_start(wk_sb[:], wk[l].rearrange("(kc p) n -> p kc n", p=P))
                    wv_sb = wa.tile([P, EC, E], BF16, tag="wv")
                    nc.sync.dma_start(wv_sb[:], wv[l].rearrange("(kc p) n -> p kc n", p=P))
                    wo_sb = wa.tile([P, EC, E], BF16, tag="wo")
                    nc.sync.dma_start(wo_sb[:], wo[l].rearrange("(kc p) n -> p kc n", p=P))
                    w1_sb = wf.tile([P, EC, DFF], BF16, tag="w1")
                    nc.sync.dma_start(w1_sb[:], w1[l].rearrange("(kc p) n -> p kc n", p=P))
                    w2_sb = wf.tile([P, FC, E], BF16, tag="w2")
                    nc.sync.dma_start(w2_sb[:], w2[l].rearrange("(kc p) n -> p kc n", p=P))
                    bq_sb = bias.tile([P, EC], F32, tag="bq")
                    nc.sync.dma_start(bq_sb[:], bqf[l])
                    bk_sb = bias.tile([P, EC], F32, tag="bk")
                    nc.sync.dma_start(bk_sb[:], bkf[l])
                    c1_sb = bias.tile([P, FC], F32, tag="c1")
                    nc.sync.dma_start(c1_sb[:], c1f[l])
                    btm_sb = bias.tile([P, 3, E], F32, tag="btm")
                    nc.sync.dma_start(btm_sb[:], btm[l].rearrange("t p n -> p t n"))

                    for g in range(NG):
                        tts = [g * TPG + i for i in range(TPG)]

                        # -- LN1 + transpose to feature-major --
                        h_fm = grp.tile([P, EC, GROUP], BF16, tag="hfm")
                        mv_g, rs_g = _ln_stats_group(nc, stat, [x_tm[tt][:] for tt in tts])
                        xhs = []
                        for i, tt in enumerate(tts):
                            xh = tk.tile([P, E], BF16, tag="xhat")
                            _ln_apply(nc, x_tm[tt][:], xh[:], mv_g, rs_g, i)
                            xhs.append(xh)
                        for kc in range(EC):
                            ptl = pstr.tile([P, GROUP], BF16, tag="tr")
                            for i in range(TPG):
                                nc.tensor.transpose(
                                    ptl[:, i * P : (i + 1) * P],
                                    xhs[i][:, kc * P : (kc + 1) * P], id_bf[:])
                            _copy_out(nc, h_fm[:, kc, :], ptl[:], _tctr)

                        # -- Q, K projections (feature-major out) --
                        q_fm = grp.tile([P, EC, GROUP], BF16, tag="qfm")
                        k_fm = grp.tile([P, EC, GROUP], BF16, tag="kfm")
                        for dst, wsb, bsb in ((q_fm, wq_sb, bq_sb), (k_fm, wk_sb, bk_sb)):
                            for m in range(EC):
                                pq = psmm.tile([P, GROUP], F32, tag="mm")
                                for kc in range(EC):
                                    nc.tensor.matmul(pq[:], wsb[:, kc, m * P : (m + 1) * P],
                                                     h_fm[:, kc, :],
                                                     start=(kc == 0), stop=(kc == EC - 1))
                                if m % 2 == 0:
                                    nc.vector.tensor_scalar(dst[:, m, :], pq[:],
                                                            bsb[:, m : m + 1], None,
                                                            op0=OP.add)
                                else:
                                    nc.scalar.activation(dst[:, m, :], pq[:], AF.Identity,
                                                         bias=bsb[:, m : m + 1], scale=1.0)

                        # -- V projection (token-major out) --
                        v_tiles = []
                        for i, tt in enumerate(tts):
                            pv = psmm.tile([P, GROUP], F32, tag="mm")
                            for kc in range(EC):
                                nc.tensor.matmul(pv[:, :E], h_fm[:, kc, i * P : (i + 1) * P],
                                                 wv_sb[:, kc, :],
                                                 start=(kc == 0), stop=(kc == EC - 1))
                            vt_i = vt.tile([P, E], BF16, tag="vtm")
                            if zero_bias:
                                nc.vector.tensor_copy(vt_i[:], pv[:, :E])
                            else:
                                nc.vector.tensor_tensor(vt_i[:], pv[:, :E], btm_sb[:, 0, :], OP.add)
                            v_tiles.append(vt_i)

                        # -- attention --
                        o_fm = grp.tile([P, EC, GROUP], BF16, tag="ofm")
                        for lb in range(BPG):
                            v0 = v_tiles[2 * lb]
                            v1 = v_tiles[2 * lb + 1]
                            for j in range(EC):  # head pair -> o_fm chunk j
                                pav = psav.tile([P, T], F32, tag="av")
                                pta = pstr.tile([P, 6, P], BF16, tag="tr")
                                for hh in range(2):
                                    h = 2 * j + hh
                                    ro = (h % 2) * 64
                                    mc = h // 2
                                    q_ap = q_fm[ro : ro + 64, mc, lb * T : (lb + 1) * T]
                                    k_ap = k_fm[ro : ro + 64, mc, lb * T : (lb + 1) * T]

                                    # scores: q-tile0 (keys 0:128) at cols 0:128,
                                    # q-tile1 (keys 0:256) at cols 128:384
                                    ps = pss.tile([P, 3 * P], F32, tag="s")
                                    nc.tensor.matmul(ps[:, :P], q_ap[:, 0:P], k_ap[:, 0:P],
                                                     start=True, stop=False)
                                    nc.tensor.matmul(ps[:, :P], id_bf[:], mask_sb[:],
                                                     start=False, stop=True)
                                    nc.tensor.matmul(ps[:, P : P + T], q_ap[:, P:T], k_ap[:],
                                                     start=True, stop=False)
                                    nc.tensor.matmul(ps[:, 2 * P : 3 * P], id_bf[:], mask_sb[:],
                                                     start=False, stop=True)
                                    p0 = bh.tile([P, P], F32, tag="p0")
                                    sum0 = stat.tile([P, 1], F32, tag="sum0")
                                    nc.scalar.activation(p0[:], ps[:, :P], AF.Exp, bias=0.0,
                                                         scale=1.0, accum_out=sum0)
                                    p0b = bh.tile([P, P], BF16, tag="p0b")
                                    if pool_norm:
                                        nc.gpsimd.normalize_recip(p0b[:], p0[:], sum0)
                                    else:
                                        r0 = stat.tile([P, 1], F32, tag="r0")
                                        nc.vector.reciprocal(r0, sum0)
                                        nc.vector.tensor_scalar_mul(p0b[:], p0[:], r0)
                                    nc.tensor.transpose(pta[:, 3 * hh, :], p0b[:], id_bf[:])

                                    p1 = bh.tile([P, T], F32, tag="p1")
                                    sum1 = stat.tile([P, 1], F32, tag="sum1")
                                    nc.scalar.activation(p1[:], ps[:, P : 3 * P], AF.Exp, bias=0.0,
                                                         scale=1.0, accum_out=sum1)
                                    p1b = bh.tile([P, T], BF16, tag="p1b")
                                    if pool_norm:
                                        nc.gpsimd.normalize_recip(p1b[:], p1[:], sum1)
                                    else:
                                        r1 = stat.tile([P, 1], F32, tag="r1")
                                        nc.vector.reciprocal(r1, sum1)
                                        nc.vector.tensor_scalar_mul(p1b[:], p1[:], r1)
                                    for kc in range(2):
                                        nc.tensor.transpose(
                                            pta[:, 3 * hh + 1 + kc, :],
                                            p1b[:, kc * P : (kc + 1) * P], id_bf[:])

                                pt_sb = bh.tile([P, 6, P], BF16, tag="ptsb")
                                for hh in range(2):
                                    h = 2 * j + hh
                                    ro = (h % 2) * 64
                                    vsl = slice(h * 64, (h + 1) * 64)
                                    _copy_out(nc, pt_sb[:, 3 * hh : 3 * hh + 3, :],
                                              pta[:, 3 * hh : 3 * hh + 3, :], _tctr)
                                    nc.tensor.matmul(pav[ro : ro + 64, 0:P], v0[:, vsl],
                                                     pt_sb[:, 3 * hh, :], start=True, stop=True)
                                    nc.tensor.matmul(pav[ro : ro + 64, P:T], v0[:, vsl],
                                                     pt_sb[:, 3 * hh + 1, :], start=True, stop=False)
                                    nc.tensor.matmul(pav[ro : ro + 64, P:T], v1[:, vsl],
                                                     pt_sb[:, 3 * hh + 2, :], start=False, stop=True)
                                _copy_out(nc, o_fm[:, j, lb * T : (lb + 1) * T], pav[:], _tctr)

                        # -- attention out-proj + bias + residual --
                        for i, tt in enumerate(tts):
                            pao = psmm.tile([P, GROUP], F32, tag="mm")
                            for kc in range(EC):
                                nc.tensor.matmul(pao[:, :E], o_fm[:, kc, i * P : (i + 1) * P],
                                                 wo_sb[:, kc, :],
                                                 start=(kc == 0), stop=(kc == EC - 1))
                            if zero_bias:
                                nc.vector.tensor_tensor(x_tm[tt][:], pao[:, :E], x_tm[tt][:], OP.add)
                            else:
                                t1 = tk.tile([P, E], F32, tag="t1")
                                nc.vector.tensor_tensor(t1[:], pao[:, :E], x_tm[tt][:], OP.add)
                                nc.gpsimd.tensor_tensor(x_tm[tt][:], t1[:], btm_sb[:, 1, :], OP.add)

                        # -- LN2 + transpose --
                        h2_fm = grp.tile([P, EC, GROUP], BF16, tag="hfm")
                        mv_g2, rs_g2 = _ln_stats_group(nc, stat, [x_tm[tt][:] for tt in tts])
                        xh2s = []
                        for i, tt in enumerate(tts):
                            xh2 = tk.tile([P, E], BF16, tag="xhat")
                            _ln_apply(nc, x_tm[tt][:], xh2[:], mv_g2, rs_g2, i)
                            xh2s.append(xh2)
                        for kc in range(EC):
                            ptl = pstr.tile([P, GROUP], BF16, tag="tr")
                            for i in range(TPG):
                                nc.tensor.transpose(
                                    ptl[:, i * P : (i + 1) * P],
                                    xh2s[i][:, kc * P : (kc + 1) * P], id_bf[:])
                            _copy_out(nc, h2_fm[:, kc, :], ptl[:], _tctr)

                        # -- FFN: W1 + relu (feature-major hidden) --
                        hf = grp1.tile([P, FC, GROUP], BF16, tag="hf")
                        for m in range(FC):
                            pf = psmm.tile([P, GROUP], F32, tag="mm")
                            for kc in range(EC):
                                nc.tensor.matmul(pf[:], w1_sb[:, kc, m * P : (m + 1) * P],
                                                 h2_fm[:, kc, :],
                                                 start=(kc == 0), stop=(kc == EC - 1))
                            if m % 2 == 0:
                                nc.vector.tensor_scalar(hf[:, m, :], pf[:],
                                                        c1_sb[:, m : m + 1], 0.0,
                                                        op0=OP.add, op1=OP.max)
                            else:
                                nc.scalar.activation(hf[:, m, :], pf[:], AF.Relu,
                                                     bias=c1_sb[:, m : m + 1], scale=1.0)

                        # -- W2 + bias + residual --
                        for i, tt in enumerate(tts):
                            pw2 = psmm.tile([P, GROUP], F32, tag="mm")
                            for kc in range(FC):
                                nc.tensor.matmul(pw2[:, :E], hf[:, kc, i * P : (i + 1) * P],
                                                 w2_sb[:, kc, :],
                                                 start=(kc == 0), stop=(kc == FC - 1))
                            if zero_bias:
                                nc.vector.tensor_tensor(x_tm[tt][:], pw2[:, :E], x_tm[tt][:], OP.add)
                            else:
                                t2 = tk.tile([P, E], F32, tag="t1")
                                nc.vector.tensor_tensor(t2[:], pw2[:, :E], x_tm[tt][:], OP.add)
                                nc.gpsimd.tensor_tensor(x_tm[tt][:], t2[:], btm_sb[:, 2, :], OP.add)

                # ---- final logits ----
                for tt in range(NT):
                    xb = tk.tile([P, E], BF16, tag="xhat")
                    nc.any.tensor_copy(out=xb[:], in_=x_tm[tt][:])
                    xf = tk.tile([P, EC, P], BF16, tag="xf")
                    ptl = pstr.tile([P, GROUP], BF16, tag="tr")
                    for kc in range(EC):
                        nc.tensor.transpose(ptl[:, kc * P : (kc + 1) * P],
                                            xb[:, kc * P : (kc + 1) * P], id_bf[:])
                    _copy_out(nc, xf[:], ptl[:, : EC * P], _tctr)
                    pl = psmm.tile([P, GROUP], F32, tag="mm")
                    for kc in range(EC):
                        nc.tensor.matmul(pl[:, :V], xf[:, kc, :], wl_sb[:, kc, :],
                                         start=(kc == 0), stop=(kc == EC - 1))
                    lg = tk.tile([P, V], F32, tag="lg")
                    if zero_bias:
                        nc.vector.tensor_copy(lg[:], pl[:, :V])
                    else:
                        nc.vector.tensor_tensor(lg[:], pl[:, :V], blr_sb[:], OP.add)
                    nc.sync.dma_start(out[tt * P : (tt + 1) * P, :], lg[:])

    nc.compile()
    return nc


def _get_prog():
    global _PROG
    if _PROG is None:
        _PROG = (build_program(), True)
    return _PROG[0]


def _prep_host(inputs):
    f32 = np.float32
    bf16 = ml_dtypes.bfloat16
    tokens = np.asarray(inputs["tokens"]).astype(np.int64)
    emb = np.asarray(inputs["emb"], dtype=f32)
    pos_enc = np.asarray(inputs["pos_enc"], dtype=f32)
    Wq = np.asarray(inputs["Wq"], dtype=f32)
    Wk = np.asarray(inputs["Wk"], dtype=f32)
    Wv = np.asarray(inputs["Wv"], dtype=f32)
    Wo = np.asarray(inputs["Wo"], dtype=f32)
    W1 = np.asarray(inputs["W1"], dtype=f32)
    W2 = np.asarray(inputs["W2"], dtype=f32)
    Wl = np.asarray(inputs["Wl"], dtype=f32)
    bq = np.asarray(inputs["bq"], dtype=f32)
    bk = np.asarray(inputs["bk"], dtype=f32)
    bv = np.asarray(inputs["bv"], dtype=f32)
    bo = np.asarray(inputs["bo"], dtype=f32)
    c1 = np.asarray(inputs["c1"], dtype=f32)
    c2 = np.asarray(inputs["c2"], dtype=f32)
    bl = np.asarray(inputs["bl"], dtype=f32)
    g1 = np.asarray(inputs["ln1_g"], dtype=f32)
    b1 = np.asarray(inputs["ln1_b"], dtype=f32)
    g2 = np.asarray(inputs["ln2_g"], dtype=f32)
    b2 = np.asarray(inputs["ln2_b"], dtype=f32)

    scale = D ** -0.5
    wq_f = np.empty((L, E, E), f32)
    wk_f = np.empty((L, E, E), f32)
    wv_f = np.empty((L, E, E), f32)
    w1_f = np.empty((L, E, DFF), f32)
    bq_f = np.empty((L, E), f32)
    bk_f = np.empty((L, E), f32)
    bv_f = np.empty((L, E), f32)
    c1_f = np.empty((L, DFF), f32)
    for l in range(L):
        wq_f[l] = g1[l][:, None] * Wq[l] * scale
        bq_f[l] = (b1[l] @ Wq[l] + bq[l]) * scale
        wk_f[l] = g1[l][:, None] * Wk[l]
        bk_f[l] = b1[l] @ Wk[l] + bk[l]
        wv_f[l] = g1[l][:, None] * Wv[l]
        bv_f[l] = b1[l] @ Wv[l] + bv[l]
        w1_f[l] = g2[l][:, None] * W1[l]
        c1_f[l] = b2[l] @ W1[l] + c1[l]

    common = {
        "embp": np.zeros((P, E), bf16),
        "pose": pos_enc,
        "maskd": np.where(np.tril(np.ones((P, P), bool)), 0.0, NEG).astype(bf16),
        "wq": wq_f.astype(bf16),
        "wk": wk_f.astype(bf16),
        "wv": wv_f.astype(bf16),
        "wo": Wo.astype(bf16),
        "w1": w1_f.astype(bf16),
        "w2": W2.astype(bf16),
        "wl": Wl.astype(bf16),
        "bqf": np.ascontiguousarray(bq_f.reshape(L, EC, P).transpose(0, 2, 1)),
        "bkf": np.ascontiguousarray(bk_f.reshape(L, EC, P).transpose(0, 2, 1)),
        "c1f": np.ascontiguousarray(c1_f.reshape(L, FC, P).transpose(0, 2, 1)),
        "btm": np.ascontiguousarray(
            np.broadcast_to(
                np.stack([bv_f, bo, c2], axis=1)[:, :, None, :], (L, 3, P, E)
            )
        ).astype(f32),
        "blr": np.broadcast_to(bl[None, :], (P, V)).astype(f32),
    }
    common["embp"][:V, :] = emb.astype(bf16)

    in_maps = []
    for c in range(N_CORES):
        tok_c = tokens[c * B_LOC : (c + 1) * B_LOC].reshape(-1)
        oht = np.zeros((P, NTOK), bf16)
        oht[tok_c, np.arange(NTOK)] = 1
        m = dict(common)
        m["oht"] = oht
        in_maps.append(m)
    return in_maps


def _biases_all_zero(inputs):
    f32 = np.float32
    zs = [inputs[k] for k in ("bq", "bk", "bv", "bo", "c1", "c2", "bl",
                              "ln1_b", "ln2_b")]
    return all(not np.any(np.asarray(z)) for z in zs)


def kernel(**inputs) -> np.ndarray:
    global _PROG
    zb = _biases_all_zero(inputs)
    if _PROG is None or _PROG[1] != zb:
        _PROG = (build_program(zero_bias=zb), zb)
    nc = _PROG[0]
    in_maps = _prep_host(inputs)
    res = run_bass_kernel_spmd(nc, in_maps, list(range(N_CORES)))
    outs = [res.results[c]["out"].reshape(B_LOC, T, V) for c in range(N_CORES)]
    return np.concatenate(outs, axis=0).astype(np.float32)



# revision 3
# speedup vs baseline: 1.6723x; 1.6723x over previous
"""Trainium2 Bass kernel for a 6-layer causal decoder transformer.

Model: B=128, T=256, E=384, H=6, D=64, DFF=1536, L=6, V=65 (f32 reference).
Sharding: pure data-parallel over batch across 8 NeuronCores (16 batches
per core), parameters replicated, no collectives.

Per-core device strategy (v2):
  - Residual stream x kept SBUF-resident, token-major [128 tok, 384] f32.
  - All matmul operands bf16 (PE 1 cyc/row); f32 PSUM accum; f32 residual.
  - LayerNorm affine folded into following weights host-side; attn scale
    folded into Wq. LN stats via bn_stats + batched Newton rsqrt on DVE;
    LN apply rotated across DVE/ScalarE (activation with per-partition
    scale/bias APs) to balance engines.
  - Attention computed with TRANSPOSED scores: S^T[k,q] = k^T q directly
    from feature-major q,k (no mask matmuls, no P transposes). Causal mask
    applied multiplicatively post-exp (0/1 triu mask) on the two diagonal
    blocks only, on the Pool engine (otherwise idle). Row sums obtained
    free via a ones-column appended to V in the AV matmul; softmax
    normalization deferred to the token-major AV output (per-partition
    reciprocal multiply). Head pairs occupy partition halves 0:64/64:128 so
    their K=64 score matmuls run concurrently in separate PE row-groups.
  - AV output is token-major [q, feat]; one PE transpose per (pair,
    q-tile) converts to feature-major for the Wo projection.
  - Embedding lookup as one-hot matmul (one-hot built host-side).
  - Bias adds elided when all bias inputs are zero (true for this
    problem's setup_inputs); non-zero biases fall back to a full-bias
    program variant.
"""

import sys
from contextlib import ExitStack

sys.path.insert(0, "/opt/trn_rl_repo")

import numpy as np
import ml_dtypes

import concourse.bass as bass
import concourse.bacc as bacc
import concourse.mybir as mybir
import concourse.tile as tile
from concourse.masks import make_identity
from concourse.bass_utils import run_bass_kernel_spmd

F32 = mybir.dt.float32
BF16 = mybir.dt.bfloat16
AF = mybir.ActivationFunctionType
OP = mybir.AluOpType
AX = mybir.AxisListType

P = 128
E, DFF, H, D, T, L, V = 384, 1536, 6, 64, 256, 6, 65
B = 128
N_CORES = 8
B_LOC = B // N_CORES          # 16 batches per core
NTOK = B_LOC * T              # 4096 tokens per core
NT = NTOK // P                # 32 token tiles
GROUP = 512                   # tokens per group (2 full batches)
NG = NTOK // GROUP            # 8 groups
TPG = GROUP // P              # 4 token tiles per group
BPG = GROUP // T              # 2 batches per group
EC = E // P                   # 3 feature chunks
FC = DFF // P                 # 12 dff chunks

_PROG = None  # (nc, zero_bias)


def _ln_stats_group(nc, stat, x_list, eps=1e-5):
    """bn_stats per tile + batched Newton rsqrt. Returns (mv_g, rs_g, mub):
    mv_g[:, i, 0:1] = mean of tile i; rs[:, i:i+1] = rsqrt(var_i + eps);
    mub[:, i:i+1] = -mean_i * rs_i (ScalarE activation bias form)."""
    n = len(x_list)
    mv_g = stat.tile([P, n, 2], F32, tag="mvg")
    for i, xin in enumerate(x_list):
        st6 = stat.tile([P, 6], F32, tag="bn6")
        nc.vector.bn_stats(out=st6[:], in_=xin)
        nc.vector.bn_aggr(out=mv_g[:, i, :], in_=st6[:])
    var = stat.tile([P, n], F32, tag="vare")
    nc.vector.tensor_scalar_add(var[:], mv_g[:, :, 1], eps)
    u = stat.tile([P, n], F32, tag="ue")
    nc.vector.reciprocal(u[:], var[:])
    lin = stat.tile([P, n], F32, tag="line")
    nc.vector.tensor_scalar(lin[:], var[:], 0.73, 0.32, op0=OP.mult, op1=OP.add)
    rs = stat.tile([P, n], F32, tag="rse")
    nc.vector.tensor_tensor(rs[:], u[:], lin[:], OP.mult)       # seed ~ rsqrt
    t1 = stat.tile([P, n], F32, tag="t1e")
    for _ in range(2):                                          # Newton x2
        nc.vector.tensor_tensor(t1[:], rs[:], rs[:], OP.mult)
        nc.vector.tensor_tensor(t1[:], t1[:], var[:], OP.mult)
        nc.vector.tensor_scalar(t1[:], t1[:], -0.5, 1.5, op0=OP.mult, op1=OP.add)
        nc.vector.tensor_tensor(rs[:], rs[:], t1[:], OP.mult)
    mub = stat.tile([P, n], F32, tag="mub")
    nc.vector.tensor_tensor(mub[:], mv_g[:, :, 0], rs[:], OP.mult)
    nc.vector.tensor_scalar(mub[:], mub[:], -1.0, None, op0=OP.mult)
    return mv_g, rs, mub


def build_program(repeat=1, dma_t=False, zero_bias=True, pool_norm=False):
    nc = bacc.Bacc("TRN2", target_bir_lowering=False, debug=False,
                   num_devices=N_CORES)

    # register const APs needed for float biases on ScalarE activations
    for val in (1e-5,):
        t = nc.alloc_sbuf_tensor(f"const-f32-{val}", [P, 1], F32)
        nc.gpsimd.memset(t.ap(), val)
        nc.const_aps.aps[(F32, val)] = t.ap()
    nc.all_engine_barrier()

    # ---- I/O -------------------------------------------------------------
    oht = nc.dram_tensor("oht", [P, NTOK], BF16, kind="ExternalInput").ap()
    embp = nc.dram_tensor("embp", [P, E], BF16, kind="ExternalInput").ap()
    pose = nc.dram_tensor("pose", [T, E], F32, kind="ExternalInput").ap()
    maskd = nc.dram_tensor("maskd", [P, P], BF16, kind="ExternalInput").ap()
    wq = nc.dram_tensor("wq", [L, E, E], BF16, kind="ExternalInput").ap()
    wk = nc.dram_tensor("wk", [L, E, E], BF16, kind="ExternalInput").ap()
    wv = nc.dram_tensor("wv", [L, E, E], BF16, kind="ExternalInput").ap()
    wo = nc.dram_tensor("wo", [L, E, E], BF16, kind="ExternalInput").ap()
    w1 = nc.dram_tensor("w1", [L, E, DFF], BF16, kind="ExternalInput").ap()
    w2 = nc.dram_tensor("w2", [L, DFF, E], BF16, kind="ExternalInput").ap()
    wl = nc.dram_tensor("wl", [E, V], BF16, kind="ExternalInput").ap()
    bqf = nc.dram_tensor("bqf", [L, P, EC], F32, kind="ExternalInput").ap()
    bkf = nc.dram_tensor("bkf", [L, P, EC], F32, kind="ExternalInput").ap()
    c1f = nc.dram_tensor("c1f", [L, P, FC], F32, kind="ExternalInput").ap()
    btm = nc.dram_tensor("btm", [L, 3, P, E], F32, kind="ExternalInput").ap()
    blr = nc.dram_tensor("blr", [P, V], F32, kind="ExternalInput").ap()
    out = nc.dram_tensor("out", [NTOK, V], F32, kind="ExternalOutput").ap()

    with tile.TileContext(nc) as tc, ExitStack() as es:
            ep = es.enter_context
            const = ep(tc.tile_pool(name="const", bufs=1))
            xres = ep(tc.tile_pool(name="xres", bufs=1))
            wa = ep(tc.tile_pool(name="wa", bufs=2))
            wf = ep(tc.tile_pool(name="wf", bufs=2))
            bias = ep(tc.tile_pool(name="bias", bufs=2))
            grp = ep(tc.tile_pool(name="grp", bufs=2))
            grp1 = ep(tc.tile_pool(name="grp1", bufs=1))
            vt = ep(tc.tile_pool(name="vt", bufs=6))
            tk = ep(tc.tile_pool(name="tk", bufs=4))
            bh = ep(tc.tile_pool(name="bh", bufs=4))
            stat = ep(tc.tile_pool(name="stat", bufs=8))
            psmm = ep(tc.tile_pool(name="psmm", bufs=2, space="PSUM"))
            pstr = ep(tc.tile_pool(name="pstr", bufs=1, space="PSUM"))
            pss = ep(tc.tile_pool(name="pss", bufs=3, space="PSUM"))
            psav = ep(tc.tile_pool(name="psav", bufs=2, space="PSUM"))
            # ---- constants ----
            id_bf = const.tile([P, P], BF16, tag="id_bf")
            make_identity(nc, id_bf)
            mask_sb = const.tile([P, P], BF16, tag="mask")   # 0/1 triu (k<=q)
            nc.sync.dma_start(mask_sb[:], maskd[:])
            emb_sb = const.tile([P, E], BF16, tag="emb")
            nc.sync.dma_start(emb_sb[:], embp[:])
            pose_sb = const.tile([P, 2, E], F32, tag="pose")
            nc.sync.dma_start(pose_sb[:, 0, :], pose[0:P, :])
            nc.sync.dma_start(pose_sb[:, 1, :], pose[P : 2 * P, :])
            wl_sb = const.tile([P, EC, V], BF16, tag="wl")
            nc.sync.dma_start(wl_sb[:], wl.rearrange("(kc p) n -> p kc n", p=P))
            blr_sb = const.tile([P, V], F32, tag="blr")
            nc.sync.dma_start(blr_sb[:], blr[:])
            oht_sb = const.tile([P, NTOK], BF16, tag="oht")
            nc.sync.dma_start(oht_sb[:], oht[:])

            _ctr = [0]

            def copy_out(dst, src):
                """PSUM->SBUF copy, alternating DVE / ScalarE."""
                if _ctr[0] % 2 == 0:
                    nc.vector.tensor_copy(dst, src)
                else:
                    nc.scalar.copy(dst, src)
                _ctr[0] += 1

            _lctr = [0]

            def ln_apply(xin, xhat_out, mv_g, rs, mub, i):
                """xhat = (x - mu) * r, rotated DVE/ScalarE."""
                if _lctr[0] % 2 == 0:
                    nc.vector.tensor_scalar(
                        xhat_out, xin, mv_g[:, i, 0:1], rs[:, i : i + 1],
                        op0=OP.subtract, op1=OP.mult)
                else:
                    nc.scalar.activation(
                        xhat_out, xin, AF.Identity,
                        bias=mub[:, i : i + 1], scale=rs[:, i : i + 1])
                _lctr[0] += 1

            x_tm = [xres.tile([P, E], F32, tag=f"x{t}", name=f"x{t}") for t in range(NT)]
            for _rep in range(repeat):
                # ---- x0 = onehot @ emb + pos ----
                for tt in range(NT):
                    xt = x_tm[tt]
                    pe = psmm.tile([P, GROUP], F32, tag="mm")
                    nc.tensor.matmul(pe[:, :E], oht_sb[:, tt * P : (tt + 1) * P],
                                     emb_sb[:], start=True, stop=True)
                    nc.vector.tensor_tensor(xt[:], pe[:, :E], pose_sb[:, tt % 2, :], OP.add)

                # ---- layers ----
                for l in range(L):
                    wq_sb = wa.tile([P, EC, E], BF16, tag="wq")
                    nc.sync.dma_start(wq_sb[:], wq[l].rearrange("(kc p) n -> p kc n", p=P))
                    wk_sb = wa.tile([P, EC, E], BF16, tag="wk")
                    nc.sync.dma_start(wk_sb[:], wk[l].rearrange("(kc p) n -> p kc n", p=P))
                    wv_sb = wa.tile([P, EC, E], BF16, tag="wv")
                    nc.sync.dma_start(wv_sb[:], wv[l].rearrange("(kc p) n -> p kc n", p=P))
                    wo_sb = wa.tile([P, EC, E], BF16, tag="wo")
                    nc.sync.dma_start(wo_sb[:], wo[l].rearrange("(kc p) n -> p kc n", p=P))
                    w1_sb = wf.tile([P, EC, DFF], BF16, tag="w1")
                    nc.sync.dma_start(w1_sb[:], w1[l].rearrange("(kc p) n -> p kc n", p=P))
                    w2_sb = wf.tile([P, FC, E], BF16, tag="w2")
                    nc.sync.dma_start(w2_sb[:], w2[l].rearrange("(kc p) n -> p kc n", p=P))
                    bq_sb = bias.tile([P, EC], F32, tag="bq")
                    nc.sync.dma_start(bq_sb[:], bqf[l])
                    bk_sb = bias.tile([P, EC], F32, tag="bk")
                    nc.sync.dma_start(bk_sb[:], bkf[l])
                    c1_sb = bias.tile([P, FC], F32, tag="c1")
                    nc.sync.dma_start(c1_sb[:], c1f[l])
                    btm_sb = bias.tile([P, 3, E], F32, tag="btm")
                    nc.sync.dma_start(btm_sb[:], btm[l].rearrange("t p n -> p t n"))

                    for g in range(NG):
                        tts = [g * TPG + i for i in range(TPG)]

                        # -- LN1 + transpose to feature-major --
                        h_fm = grp.tile([P, EC, GROUP], BF16, tag="hfm")
                        mv_g, rs_g, mub_g = _ln_stats_group(
                            nc, stat, [x_tm[tt][:] for tt in tts])
                        xhs = []
                        for i, tt in enumerate(tts):
                            xh = tk.tile([P, E], BF16, tag="xhat")
                            ln_apply(x_tm[tt][:], xh[:], mv_g, rs_g, mub_g, i)
                            xhs.append(xh)
                        for kc in range(EC):
                            ptl = pstr.tile([P, GROUP], BF16, tag="tr")
                            for i in range(TPG):
                                nc.tensor.transpose(
                                    ptl[:, i * P : (i + 1) * P],
                                    xhs[i][:, kc * P : (kc + 1) * P], id_bf[:])
                            copy_out(h_fm[:, kc, :], ptl[:])

                        # -- Q, K projections (feature-major out) --
                        q_fm = grp.tile([P, EC, GROUP], BF16, tag="qfm")
                        k_fm = grp.tile([P, EC, GROUP], BF16, tag="kfm")
                        for dst, wsb, bsb in ((q_fm, wq_sb, bq_sb), (k_fm, wk_sb, bk_sb)):
                            for m in range(EC):
                                pq = psmm.tile([P, GROUP], F32, tag="mm")
                                for kc in range(EC):
                                    nc.tensor.matmul(pq[:], wsb[:, kc, m * P : (m + 1) * P],
                                                     h_fm[:, kc, :],
                                                     start=(kc == 0), stop=(kc == EC - 1))
                                if zero_bias:
                                    copy_out(dst[:, m, :], pq[:])
                                elif m % 2 == 0:
                                    nc.vector.tensor_scalar(dst[:, m, :], pq[:],
                                                            bsb[:, m : m + 1], None,
                                                            op0=OP.add)
                                else:
                                    nc.scalar.activation(dst[:, m, :], pq[:], AF.Identity,
                                                         bias=bsb[:, m : m + 1], scale=1.0)

                        # -- V projection (token-major out, +ones column) --
                        v_tiles = []
                        for i, tt in enumerate(tts):
                            pv = psmm.tile([P, GROUP], F32, tag="mm")
                            for kc in range(EC):
                                nc.tensor.matmul(pv[:, :E], h_fm[:, kc, i * P : (i + 1) * P],
                                                 wv_sb[:, kc, :],
                                                 start=(kc == 0), stop=(kc == EC - 1))
                            vt_i = vt.tile([P, H, D + 1], BF16, tag="vtm")
                            if zero_bias:
                                copy_out(vt_i[:, :, 0:D], pv[:, :E])
                            else:
                                nc.vector.tensor_tensor(vt_i[:, :, 0:D], pv[:, :E],
                                                        btm_sb[:, 0, :], OP.add)
                            nc.gpsimd.memset(vt_i[:, :, D : D + 1], 1.0)
                            v_tiles.append(vt_i)

                        # -- attention (transposed scores) --
                        o_fm = grp.tile([P, EC, GROUP], BF16, tag="ofm")
                        for j in range(EC):          # head pair (2j, 2j+1)
                            ptl = pstr.tile([P, GROUP], BF16, tag="tr")
                            for lb in range(BPG):
                                v0 = v_tiles[2 * lb]
                                v1 = v_tiles[2 * lb + 1]
                                q0 = lb * T
                                # scores S^T: [keys, queries]; head pair in
                                # partition halves -> concurrent PE row-groups
                                pts = [
                                    pss.tile([P, EC * P], F32, tag="sT",
                                             name=f"sT{hh}")
                                    for hh in range(2)
                                ]
                                for kt in range(2):
                                    for hh in range(2):
                                        ro = hh * D
                                        if kt == 0:
                                            nc.tensor.matmul(
                                                pts[hh][:, 0:T],
                                                k_fm[ro : ro + D, j, q0 : q0 + P],
                                                q_fm[ro : ro + D, j, q0 : q0 + T],
                                                start=True, stop=True)
                                        else:
                                            nc.tensor.matmul(
                                                pts[hh][:, T : T + P],
                                                k_fm[ro : ro + D, j, q0 + P : q0 + T],
                                                q_fm[ro : ro + D, j, q0 + P : q0 + T],
                                                start=True, stop=True)
                                # exp (no accum; sums come from ones column)
                                pbs = []
                                for hh in range(2):
                                    pb = bh.tile([P, EC * P], BF16, tag="pb")
                                    nc.scalar.activation(pb[:], pts[hh][:], AF.Exp,
                                                         bias=0.0, scale=1.0)
                                    # causal 0/1 mask on the two diagonal
                                    # blocks (Pool engine)
                                    nc.gpsimd.tensor_tensor(
                                        pb[:, 0:P], pb[:, 0:P], mask_sb[:], OP.mult)
                                    nc.gpsimd.tensor_tensor(
                                        pb[:, 2 * P : 3 * P], pb[:, 2 * P : 3 * P],
                                        mask_sb[:], OP.mult)
                                    pbs.append(pb)
                                # AV (token-major out) + ones-column sums
                                pav = psav.tile([P, 4, D + 1], F32, tag="av")
                                for hh in range(2):
                                    h = 2 * j + hh
                                    pb = pbs[hh]
                                    nc.tensor.matmul(pav[:, 2 * hh, :],
                                                     pb[:, 0:P], v0[:, h, :],
                                                     start=True, stop=True)
                                    nc.tensor.matmul(pav[:, 2 * hh + 1, :],
                                                     pb[:, P:T], v0[:, h, :],
                                                     start=True, stop=False)
                                    nc.tensor.matmul(pav[:, 2 * hh + 1, :],
                                                     pb[:, T : T + P], v1[:, h, :],
                                                     start=False, stop=True)
                                # normalize: o = av / sums (per-partition)
                                rr = stat.tile([P, 4], F32, tag="rr")
                                nc.vector.reciprocal(rr[:], pav[:, :, D : D + 1])
                                o_tm = tk.tile([P, 2, P], BF16, tag="otm")
                                for hh in range(2):
                                    for qt in range(2):
                                        s = 2 * hh + qt
                                        dst = o_tm[:, qt, hh * D : (hh + 1) * D]
                                        if s % 2 == 0:
                                            nc.vector.tensor_scalar(
                                                dst, pav[:, s, 0:D],
                                                rr[:, s : s + 1], None, op0=OP.mult)
                                        else:
                                            nc.scalar.activation(
                                                dst, pav[:, s, 0:D], AF.Identity,
                                                bias=0.0, scale=rr[:, s : s + 1])
                                # transpose o to feature-major
                                for qt in range(2):
                                    nc.tensor.transpose(
                                        ptl[:, (2 * lb + qt) * P : (2 * lb + qt + 1) * P],
                                        o_tm[:, qt, :], id_bf[:])
                            copy_out(o_fm[:, j, :], ptl[:])

                        # -- attention out-proj + residual --
                        for i, tt in enumerate(tts):
                            pao = psmm.tile([P, GROUP], F32, tag="mm")
                            for kc in range(EC):
                                nc.tensor.matmul(pao[:, :E], o_fm[:, kc, i * P : (i + 1) * P],
                                                 wo_sb[:, kc, :],
                                                 start=(kc == 0), stop=(kc == EC - 1))
                            if zero_bias:
                                nc.vector.tensor_tensor(x_tm[tt][:], pao[:, :E], x_tm[tt][:], OP.add)
                            else:
                                t1 = tk.tile([P, E], F32, tag="t1")
                                nc.vector.tensor_tensor(t1[:], pao[:, :E], x_tm[tt][:], OP.add)
                                nc.gpsimd.tensor_tensor(x_tm[tt][:], t1[:], btm_sb[:, 1, :], OP.add)

                        # -- LN2 + transpose --
                        h2_fm = grp.tile([P, EC, GROUP], BF16, tag="hfm")
                        mv_g2, rs_g2, mub_g2 = _ln_stats_group(
                            nc, stat, [x_tm[tt][:] for tt in tts])
                        xh2s = []
                        for i, tt in enumerate(tts):
                            xh2 = tk.tile([P, E], BF16, tag="xhat")
                            ln_apply(x_tm[tt][:], xh2[:], mv_g2, rs_g2, mub_g2, i)
                            xh2s.append(xh2)
                        for kc in range(EC):
                            ptl = pstr.tile([P, GROUP], BF16, tag="tr")
                            for i in range(TPG):
                                nc.tensor.transpose(
                                    ptl[:, i * P : (i + 1) * P],
                                    xh2s[i][:, kc * P : (kc + 1) * P], id_bf[:])
                            copy_out(h2_fm[:, kc, :], ptl[:])

                        # -- FFN: W1 + relu (feature-major hidden) --
                        hf = grp1.tile([P, FC, GROUP], BF16, tag="hf")
                        for m in range(FC):
                            pf = psmm.tile([P, GROUP], F32, tag="mm")
                            for kc in range(EC):
                                nc.tensor.matmul(pf[:], w1_sb[:, kc, m * P : (m + 1) * P],
                                                 h2_fm[:, kc, :],
                                                 start=(kc == 0), stop=(kc == EC - 1))
                            if zero_bias:
                                if m % 2 == 0:
                                    nc.vector.tensor_scalar(hf[:, m, :], pf[:],
                                                            0.0, None, op0=OP.max)
                                else:
                                    nc.scalar.activation(hf[:, m, :], pf[:], AF.Relu,
                                                         bias=0.0, scale=1.0)
                            elif m % 2 == 0:
                                nc.vector.tensor_scalar(hf[:, m, :], pf[:],
                                                        c1_sb[:, m : m + 1], 0.0,
                                                        op0=OP.add, op1=OP.max)
                            else:
                                nc.scalar.activation(hf[:, m, :], pf[:], AF.Relu,
                                                     bias=c1_sb[:, m : m + 1], scale=1.0)

                        # -- W2 + residual --
                        for i, tt in enumerate(tts):
                            pw2 = psmm.tile([P, GROUP], F32, tag="mm")
                            for kc in range(FC):
                                nc.tensor.matmul(pw2[:, :E], hf[:, kc, i * P : (i + 1) * P],
                                                 w2_sb[:, kc, :],
                                                 start=(kc == 0), stop=(kc == FC - 1))
                            if zero_bias:
                                nc.vector.tensor_tensor(x_tm[tt][:], pw2[:, :E], x_tm[tt][:], OP.add)
                            else:
                                t2 = tk.tile([P, E], F32, tag="t1")
                                nc.vector.tensor_tensor(t2[:], pw2[:, :E], x_tm[tt][:], OP.add)
                                nc.gpsimd.tensor_tensor(x_tm[tt][:], t2[:], btm_sb[:, 2, :], OP.add)

                # ---- final logits ----
                for tt in range(NT):
                    xb = tk.tile([P, E], BF16, tag="xhat")
                    nc.any.tensor_copy(out=xb[:], in_=x_tm[tt][:])
                    xf = tk.tile([P, EC, P], BF16, tag="xf")
                    ptl = pstr.tile([P, GROUP], BF16, tag="tr")
                    for kc in range(EC):
                        nc.tensor.transpose(ptl[:, kc * P : (kc + 1) * P],
                                            xb[:, kc * P : (kc + 1) * P], id_bf[:])
                    copy_out(xf[:], ptl[:, : EC * P])
                    pl = psmm.tile([P, GROUP], F32, tag="mm")
                    for kc in range(EC):
                        nc.tensor.matmul(pl[:, :V], xf[:, kc, :], wl_sb[:, kc, :],
                                         start=(kc == 0), stop=(kc == EC - 1))
                    lg = tk.tile([P, V], F32, tag="lg")
                    if zero_bias:
                        nc.vector.tensor_copy(lg[:], pl[:, :V])
                    else:
                        nc.vector.tensor_tensor(lg[:], pl[:, :V], blr_sb[:], OP.add)
                    nc.sync.dma_start(out[tt * P : (tt + 1) * P, :], lg[:])

    nc.compile()
    return nc


def _prep_host(inputs):
    f32 = np.float32
    bf16 = ml_dtypes.bfloat16
    tokens = np.asarray(inputs["tokens"]).astype(np.int64)
    emb = np.asarray(inputs["emb"], dtype=f32)
    pos_enc = np.asarray(inputs["pos_enc"], dtype=f32)
    Wq = np.asarray(inputs["Wq"], dtype=f32)
    Wk = np.asarray(inputs["Wk"], dtype=f32)
    Wv = np.asarray(inputs["Wv"], dtype=f32)
    Wo = np.asarray(inputs["Wo"], dtype=f32)
    W1 = np.asarray(inputs["W1"], dtype=f32)
    W2 = np.asarray(inputs["W2"], dtype=f32)
    Wl = np.asarray(inputs["Wl"], dtype=f32)
    bq = np.asarray(inputs["bq"], dtype=f32)
    bk = np.asarray(inputs["bk"], dtype=f32)
    bv = np.asarray(inputs["bv"], dtype=f32)
    bo = np.asarray(inputs["bo"], dtype=f32)
    c1 = np.asarray(inputs["c1"], dtype=f32)
    c2 = np.asarray(inputs["c2"], dtype=f32)
    bl = np.asarray(inputs["bl"], dtype=f32)
    g1 = np.asarray(inputs["ln1_g"], dtype=f32)
    b1 = np.asarray(inputs["ln1_b"], dtype=f32)
    g2 = np.asarray(inputs["ln2_g"], dtype=f32)
    b2 = np.asarray(inputs["ln2_b"], dtype=f32)

    scale = D ** -0.5
    wq_f = np.empty((L, E, E), f32)
    wk_f = np.empty((L, E, E), f32)
    wv_f = np.empty((L, E, E), f32)
    w1_f = np.empty((L, E, DFF), f32)
    bq_f = np.empty((L, E), f32)
    bk_f = np.empty((L, E), f32)
    bv_f = np.empty((L, E), f32)
    c1_f = np.empty((L, DFF), f32)
    for l in range(L):
        wq_f[l] = g1[l][:, None] * Wq[l] * scale
        bq_f[l] = (b1[l] @ Wq[l] + bq[l]) * scale
        wk_f[l] = g1[l][:, None] * Wk[l]
        bk_f[l] = b1[l] @ Wk[l] + bk[l]
        wv_f[l] = g1[l][:, None] * Wv[l]
        bv_f[l] = b1[l] @ Wv[l] + bv[l]
        w1_f[l] = g2[l][:, None] * W1[l]
        c1_f[l] = b2[l] @ W1[l] + c1[l]

    common = {
        "embp": np.zeros((P, E), bf16),
        "pose": pos_enc,
        # 0/1 mask, transposed-causal: keep key k for query q iff k <= q
        "maskd": np.triu(np.ones((P, P), f32)).astype(bf16),
        "wq": wq_f.astype(bf16),
        "wk": wk_f.astype(bf16),
        "wv": wv_f.astype(bf16),
        "wo": Wo.astype(bf16),
        "w1": w1_f.astype(bf16),
        "w2": W2.astype(bf16),
        "wl": Wl.astype(bf16),
        "bqf": np.ascontiguousarray(bq_f.reshape(L, EC, P).transpose(0, 2, 1)),
        "bkf": np.ascontiguousarray(bk_f.reshape(L, EC, P).transpose(0, 2, 1)),
        "c1f": np.ascontiguousarray(c1_f.reshape(L, FC, P).transpose(0, 2, 1)),
        "btm": np.ascontiguousarray(
            np.broadcast_to(
                np.stack([bv_f, bo, c2], axis=1)[:, :, None, :], (L, 3, P, E)
            )
        ).astype(f32),
        "blr": np.broadcast_to(bl[None, :], (P, V)).astype(f32),
    }
    common["embp"][:V, :] = emb.astype(bf16)

    in_maps = []
    for c in range(N_CORES):
        tok_c = tokens[c * B_LOC : (c + 1) * B_LOC].reshape(-1)
        oht = np.zeros((P, NTOK), bf16)
        oht[tok_c, np.arange(NTOK)] = 1
        m = dict(common)
        m["oht"] = oht
        in_maps.append(m)
    return in_maps


def _biases_all_zero(inputs):
    zs = [inputs[k] for k in ("bq", "bk", "bv", "bo", "c1", "c2", "bl",
                              "ln1_b", "ln2_b")]
    return all(not np.any(np.asarray(z)) for z in zs)


def kernel(**inputs) -> np.ndarray:
    global _PROG
    zb = _biases_all_zero(inputs)
    if _PROG is None or _PROG[1] != zb:
        _PROG = (build_program(zero_bias=zb), zb)
    nc = _PROG[0]
    in_maps = _prep_host(inputs)
    res = run_bass_kernel_spmd(nc, in_maps, list(range(N_CORES)))
    outs = [res.results[c]["out"].reshape(B_LOC, T, V) for c in range(N_CORES)]
    return np.concatenate(outs, axis=0).astype(np.float32)


# revision 22
# speedup vs baseline: 2.3901x; 1.4293x over previous
"""Trainium2 Bass kernel for a 6-layer causal decoder transformer.

Model: B=128, T=256, E=384, H=6, D=64, DFF=1536, L=6, V=65 (f32 reference).
Sharding: pure data-parallel over batch across 8 NeuronCores (16 batches
per core), parameters replicated, no collectives.

Per-core device strategy (v2):
  - Residual stream x kept SBUF-resident, token-major [128 tok, 384] f32.
  - All matmul operands bf16 (PE 1 cyc/row); f32 PSUM accum; f32 residual.
  - LayerNorm affine folded into following weights host-side; attn scale
    folded into Wq. LN stats via bn_stats + batched Newton rsqrt on DVE;
    LN apply rotated across DVE/ScalarE (activation with per-partition
    scale/bias APs) to balance engines.
  - Attention computed with TRANSPOSED scores: S^T[k,q] = k^T q directly
    from feature-major q,k (no mask matmuls, no P transposes). Causal mask
    applied multiplicatively post-exp (0/1 triu mask) on the two diagonal
    blocks only, on the Pool engine (otherwise idle). Row sums obtained
    free via a ones-column appended to V in the AV matmul; softmax
    normalization deferred to the token-major AV output (per-partition
    reciprocal multiply). Head pairs occupy partition halves 0:64/64:128 so
    their K=64 score matmuls run concurrently in separate PE row-groups.
  - AV output is token-major [q, feat]; one PE transpose per (pair,
    q-tile) converts to feature-major for the Wo projection.
  - Embedding lookup as one-hot matmul (one-hot built host-side).
  - Bias adds elided when all bias inputs are zero (true for this
    problem's setup_inputs); non-zero biases fall back to a full-bias
    program variant.
"""

import sys
from contextlib import ExitStack

sys.path.insert(0, "/opt/trn_rl_repo")

import numpy as np
import ml_dtypes

import concourse.bass as bass
import concourse.bacc as bacc
import concourse.mybir as mybir
import concourse.tile as tile
from concourse.masks import make_identity
from concourse.bass_utils import run_bass_kernel_spmd

F32 = mybir.dt.float32
BF16 = mybir.dt.bfloat16
F8 = mybir.dt.float8e4
DR = mybir.MatmulPerfMode.DoubleRow
AF = mybir.ActivationFunctionType
OP = mybir.AluOpType
AX = mybir.AxisListType

WS = 32.0            # fp8 weight pre-scale (host); descaled in consumers
IWS = 1.0 / WS

P = 128
E, DFF, H, D, T, L, V = 384, 1536, 6, 64, 256, 6, 65
B = 128
N_CORES = 8
B_LOC = B // N_CORES          # 16 batches per core
NTOK = B_LOC * T              # 4096 tokens per core
NT = NTOK // P                # 32 token tiles
GROUP = 512                   # tokens per group (2 full batches)
NG = NTOK // GROUP            # 8 groups
TPG = GROUP // P              # 4 token tiles per group
BPG = GROUP // T              # 2 batches per group
EC = E // P                   # 3 feature chunks
FC = DFF // P                 # 12 dff chunks

_PROG = None  # (nc, zero_bias)


def _ln_stats_group(nc, stat, x_list, eps=1e-5):
    """bn_stats per tile + batched Newton rsqrt. Returns (mv_g, rs_g, mub):
    mv_g[:, i, 0:1] = mean of tile i; rs[:, i:i+1] = rsqrt(var_i + eps);
    mub[:, i:i+1] = -mean_i * rs_i (ScalarE activation bias form)."""
    n = len(x_list)
    mv_g = stat.tile([P, n, 2], F32, tag="mvg")
    for i, xin in enumerate(x_list):
        st6 = stat.tile([P, 6], F32, tag="bn6")
        nc.vector.bn_stats(out=st6[:], in_=xin)
        nc.vector.bn_aggr(out=mv_g[:, i, :], in_=st6[:])
    var = stat.tile([P, n], F32, tag="vare")
    nc.vector.tensor_scalar_add(var[:], mv_g[:, :, 1], eps)
    u = stat.tile([P, n], F32, tag="ue")
    nc.vector.reciprocal(u[:], var[:])
    lin = stat.tile([P, n], F32, tag="line")
    nc.vector.tensor_scalar(lin[:], var[:], 0.73, 0.32, op0=OP.mult, op1=OP.add)
    rs = stat.tile([P, n], F32, tag="rse")
    nc.vector.tensor_tensor(rs[:], u[:], lin[:], OP.mult)       # seed ~ rsqrt
    t1 = stat.tile([P, n], F32, tag="t1e")
    for _ in range(2):                                          # Newton x2
        nc.vector.tensor_tensor(t1[:], rs[:], rs[:], OP.mult)
        nc.vector.tensor_tensor(t1[:], t1[:], var[:], OP.mult)
        nc.vector.tensor_scalar(t1[:], t1[:], -0.5, 1.5, op0=OP.mult, op1=OP.add)
        nc.vector.tensor_tensor(rs[:], rs[:], t1[:], OP.mult)
    mub = stat.tile([P, n], F32, tag="mub")
    nc.vector.tensor_tensor(mub[:], mv_g[:, :, 0], rs[:], OP.mult)
    nc.vector.tensor_scalar(mub[:], mub[:], -1.0, None, op0=OP.mult)
    return mv_g, rs, mub


def build_program(repeat=1, dma_t=False, zero_bias=True, pool_norm=False,
                  fp8=False):
    # fp8 here = fp8 WEIGHTS (QKVO/W1/W2). Measured rel err 0.049 > 2e-2 gate,
    # so off by default; the attention-probability fp8 path is always on.
    fp8 = fp8 and zero_bias
    WDT = F8 if fp8 else BF16   # big-weight / fp8-activation dtype
    nc = bacc.Bacc("TRN2", target_bir_lowering=False, debug=False,
                   num_devices=N_CORES)

    # register const APs needed for float biases on ScalarE activations
    for val in (1e-5,):
        t = nc.alloc_sbuf_tensor(f"const-f32-{val}", [P, 1], F32)
        nc.gpsimd.memset(t.ap(), val)
        nc.const_aps.aps[(F32, val)] = t.ap()
    nc.all_engine_barrier()

    # ---- I/O -------------------------------------------------------------
    oht = nc.dram_tensor("oht", [P, NTOK], BF16, kind="ExternalInput").ap()
    embp = nc.dram_tensor("embp", [P, E], BF16, kind="ExternalInput").ap()
    pose = nc.dram_tensor("pose", [T, E], F32, kind="ExternalInput").ap()
    maskd = nc.dram_tensor("maskd", [P, P], BF16, kind="ExternalInput").ap()
    wq = nc.dram_tensor("wq", [L, E, E], WDT, kind="ExternalInput").ap()
    wk = nc.dram_tensor("wk", [L, E, E], WDT, kind="ExternalInput").ap()
    wv = nc.dram_tensor("wv", [L, E, E], WDT, kind="ExternalInput").ap()
    wo = nc.dram_tensor("wo", [L, E, E], WDT, kind="ExternalInput").ap()
    w1 = nc.dram_tensor("w1", [L, E, DFF], WDT, kind="ExternalInput").ap()
    w2 = nc.dram_tensor("w2", [L, DFF, E], WDT, kind="ExternalInput").ap()
    wl = nc.dram_tensor("wl", [E, V], BF16, kind="ExternalInput").ap()
    bqf = nc.dram_tensor("bqf", [L, P, EC], F32, kind="ExternalInput").ap()
    bkf = nc.dram_tensor("bkf", [L, P, EC], F32, kind="ExternalInput").ap()
    c1f = nc.dram_tensor("c1f", [L, P, FC], F32, kind="ExternalInput").ap()
    btm = nc.dram_tensor("btm", [L, 3, P, E], F32, kind="ExternalInput").ap()
    blr = nc.dram_tensor("blr", [P, V], F32, kind="ExternalInput").ap()
    out = nc.dram_tensor("out", [NTOK, V], F32, kind="ExternalOutput").ap()

    with tile.TileContext(nc) as tc, ExitStack() as es:
            ep = es.enter_context
            const = ep(tc.tile_pool(name="const", bufs=1))
            xres = ep(tc.tile_pool(name="xres", bufs=1))
            wa = ep(tc.tile_pool(name="wa", bufs=2))
            wf = ep(tc.tile_pool(name="wf", bufs=2))
            bias = ep(tc.tile_pool(name="bias", bufs=2))
            grp = ep(tc.tile_pool(name="grp", bufs=2))
            grp1 = ep(tc.tile_pool(name="grp1", bufs=1))
            vt = ep(tc.tile_pool(name="vt", bufs=6))
            tk = ep(tc.tile_pool(name="tk", bufs=4))
            bh = ep(tc.tile_pool(name="bh", bufs=4))
            stat = ep(tc.tile_pool(name="stat", bufs=8))
            psmm = ep(tc.tile_pool(name="psmm", bufs=2, space="PSUM"))
            pstr = ep(tc.tile_pool(name="pstr", bufs=1, space="PSUM"))
            pss = ep(tc.tile_pool(name="pss", bufs=2, space="PSUM"))
            psav = ep(tc.tile_pool(name="psav", bufs=1, space="PSUM"))
            # ---- constants ----
            id_bf = const.tile([P, P], BF16, tag="id_bf")
            make_identity(nc, id_bf)
            mask_sb = const.tile([P, P], BF16, tag="mask")   # 0/1 triu (k<=q)
            nc.sync.dma_start(mask_sb[:], maskd[:])
            emb_sb = const.tile([P, E], BF16, tag="emb")
            nc.sync.dma_start(emb_sb[:], embp[:])
            pose_sb = const.tile([P, 2, E], F32, tag="pose")
            nc.sync.dma_start(pose_sb[:, 0, :], pose[0:P, :])
            nc.sync.dma_start(pose_sb[:, 1, :], pose[P : 2 * P, :])
            wl_sb = const.tile([P, EC, V], BF16, tag="wl")
            nc.sync.dma_start(wl_sb[:], wl.rearrange("(kc p) n -> p kc n", p=P))
            blr_sb = const.tile([P, V], F32, tag="blr")
            nc.sync.dma_start(blr_sb[:], blr[:])
            oht_sb = const.tile([P, NTOK], BF16, tag="oht")
            nc.sync.dma_start(oht_sb[:], oht[:])

            _ctr = [0]

            def copy_out(dst, src):
                """PSUM->SBUF copy, alternating DVE / ScalarE."""
                if _ctr[0] % 2 == 0:
                    nc.vector.tensor_copy(dst, src)
                else:
                    nc.scalar.copy(dst, src)
                _ctr[0] += 1

            _lctr = [0]

            def ln_apply(xin, xhat_out, mv_g, rs, mub, i):
                """xhat = (x - mu) * r, rotated DVE/ScalarE."""
                if _lctr[0] % 2 == 0:
                    nc.vector.tensor_scalar(
                        xhat_out, xin, mv_g[:, i, 0:1], rs[:, i : i + 1],
                        op0=OP.subtract, op1=OP.mult)
                else:
                    nc.scalar.activation(
                        xhat_out, xin, AF.Identity,
                        bias=mub[:, i : i + 1], scale=rs[:, i : i + 1])
                _lctr[0] += 1

            x_tm = [xres.tile([P, E], F32, tag=f"x{t}", name=f"x{t}") for t in range(NT)]
            for _rep in range(repeat):
                # ---- x0 = onehot @ emb + pos ----
                for tt in range(NT):
                    xt = x_tm[tt]
                    pe = psmm.tile([P, GROUP], F32, tag="mm")
                    nc.tensor.matmul(pe[:, :E], oht_sb[:, tt * P : (tt + 1) * P],
                                     emb_sb[:], start=True, stop=True)
                    nc.vector.tensor_tensor(xt[:], pe[:, :E], pose_sb[:, tt % 2, :], OP.add)

                # ---- layers ----
                for l in range(L):
                    wq_sb = wa.tile([P, EC, E], WDT, tag="wq")
                    nc.sync.dma_start(wq_sb[:], wq[l].rearrange("(kc p) n -> p kc n", p=P))
                    wk_sb = wa.tile([P, EC, E], WDT, tag="wk")
                    nc.sync.dma_start(wk_sb[:], wk[l].rearrange("(kc p) n -> p kc n", p=P))
                    wv_sb = wa.tile([P, EC, E], WDT, tag="wv")
                    nc.sync.dma_start(wv_sb[:], wv[l].rearrange("(kc p) n -> p kc n", p=P))
                    wo_sb = wa.tile([P, EC, E], WDT, tag="wo")
                    nc.sync.dma_start(wo_sb[:], wo[l].rearrange("(kc p) n -> p kc n", p=P))
                    w1_sb = wf.tile([P, EC, DFF], WDT, tag="w1")
                    nc.sync.dma_start(w1_sb[:], w1[l].rearrange("(kc p) n -> p kc n", p=P))
                    w2_sb = wf.tile([P, FC, E], WDT, tag="w2")
                    nc.sync.dma_start(w2_sb[:], w2[l].rearrange("(kc p) n -> p kc n", p=P))
                    bq_sb = bias.tile([P, EC], F32, tag="bq")
                    nc.sync.dma_start(bq_sb[:], bqf[l])
                    bk_sb = bias.tile([P, EC], F32, tag="bk")
                    nc.sync.dma_start(bk_sb[:], bkf[l])
                    c1_sb = bias.tile([P, FC], F32, tag="c1")
                    nc.sync.dma_start(c1_sb[:], c1f[l])
                    btm_sb = bias.tile([P, 3, E], F32, tag="btm")
                    nc.sync.dma_start(btm_sb[:], btm[l].rearrange("t p n -> p t n"))

                    for g in range(NG):
                        tts = [g * TPG + i for i in range(TPG)]

                        # -- LN1 + transpose to feature-major --
                        h_fm = grp.tile([P, EC, GROUP], WDT, tag="hfm")
                        mv_g, rs_g, mub_g = _ln_stats_group(
                            nc, stat, [x_tm[tt][:] for tt in tts])
                        xhs = []
                        for i, tt in enumerate(tts):
                            xh = tk.tile([P, E], BF16, tag="xhat")
                            ln_apply(x_tm[tt][:], xh[:], mv_g, rs_g, mub_g, i)
                            xhs.append(xh)
                        for kc in range(EC):
                            ptl = pstr.tile([P, GROUP], BF16, tag="tr")
                            for i in range(TPG):
                                nc.tensor.transpose(
                                    ptl[:, i * P : (i + 1) * P],
                                    xhs[i][:, kc * P : (kc + 1) * P], id_bf[:])
                            copy_out(h_fm[:, kc, :], ptl[:])

                        # -- Q, K projections (feature-major out) --
                        q_fm = grp.tile([P, EC, GROUP], BF16, tag="qfm")
                        k_fm = grp.tile([P, EC, GROUP], BF16, tag="kfm")
                        for dst, wsb, bsb in ((q_fm, wq_sb, bq_sb), (k_fm, wk_sb, bk_sb)):
                            for m in range(EC):
                                pq = psmm.tile([P, GROUP], F32, tag="mm")
                                if fp8:
                                    nc.tensor.matmul(pq[:], wsb[:, 0:2, m * P : (m + 1) * P],
                                                     h_fm[:, 0:2, :],
                                                     start=True, stop=False, perf_mode=DR)
                                    nc.tensor.matmul(pq[:], wsb[:, 2, m * P : (m + 1) * P],
                                                     h_fm[:, 2, :],
                                                     start=False, stop=True)
                                else:
                                    for kc in range(EC):
                                        nc.tensor.matmul(pq[:], wsb[:, kc, m * P : (m + 1) * P],
                                                         h_fm[:, kc, :],
                                                         start=(kc == 0), stop=(kc == EC - 1))
                                if zero_bias:
                                    copy_out(dst[:, m, :], pq[:])
                                elif m % 2 == 0:
                                    nc.vector.tensor_scalar(dst[:, m, :], pq[:],
                                                            bsb[:, m : m + 1], None,
                                                            op0=OP.add)
                                else:
                                    nc.scalar.activation(dst[:, m, :], pq[:], AF.Identity,
                                                         bias=bsb[:, m : m + 1], scale=1.0)

                        # -- V projection (token-major out, +ones column) --
                        # per-batch tiles [P, H, kt, VP]: both key-tiles of a
                        # batch interleaved so AV q-tile1 runs as ONE fp8
                        # DoubleRow matmul. VP=80 keeps the kt step %16==0.
                        VP = 80
                        v_bt = []
                        for lb in range(BPG):
                            v_b = vt.tile([P, H, 2, VP], F8, tag="vtm")
                            for t in range(2):
                                i = 2 * lb + t
                                pv = psmm.tile([P, GROUP], F32, tag="mm")
                                if fp8:
                                    nc.tensor.matmul(pv[:, :E], h_fm[:, 0:2, i * P : (i + 1) * P],
                                                     wv_sb[:, 0:2, :],
                                                     start=True, stop=False, perf_mode=DR)
                                    nc.tensor.matmul(pv[:, :E], h_fm[:, 2, i * P : (i + 1) * P],
                                                     wv_sb[:, 2, :],
                                                     start=False, stop=True)
                                else:
                                    for kc in range(EC):
                                        nc.tensor.matmul(pv[:, :E], h_fm[:, kc, i * P : (i + 1) * P],
                                                         wv_sb[:, kc, :],
                                                         start=(kc == 0), stop=(kc == EC - 1))
                                if zero_bias:
                                    copy_out(v_b[:, :, t, 0:D], pv[:, :E])
                                else:
                                    nc.vector.tensor_tensor(v_b[:, :, t, 0:D], pv[:, :E],
                                                            btm_sb[:, 0, :], OP.add)
                            # v carries the x WS weight scale when fp8w; a WS
                            # ones column makes the AV sums carry it too, so
                            # normalize cancels both exactly.
                            nc.gpsimd.memset(v_b[:, :, :, D : D + 1], WS if fp8 else 1.0)
                            v_bt.append(v_b)

                        # -- attention (transposed scores, fp8 probabilities) --
                        o_fm = grp.tile([P, EC, GROUP], WDT, tag="ofm")
                        for j in range(EC):          # head pair (2j, 2j+1)
                            ptl = pstr.tile([P, GROUP], BF16, tag="tr")
                            for lb in range(BPG):
                                v_b = v_bt[lb]
                                q0 = lb * T
                                # scores S^T: [keys, queries]; head pair in
                                # partition halves -> concurrent PE row-groups
                                psp = pss.tile([P, 2, 4 * P], F32, tag="sT")
                                for kt in range(2):
                                    for hh in range(2):
                                        ro = hh * D
                                        if kt == 0:
                                            nc.tensor.matmul(
                                                psp[:, hh, 0:T],
                                                k_fm[ro : ro + D, j, q0 : q0 + P],
                                                q_fm[ro : ro + D, j, q0 : q0 + T],
                                                start=True, stop=True)
                                        else:
                                            nc.tensor.matmul(
                                                psp[:, hh, T : T + P],
                                                k_fm[ro : ro + D, j, q0 + P : q0 + T],
                                                q_fm[ro : ro + D, j, q0 + P : q0 + T],
                                                start=True, stop=True)
                                # one exp for the pair (no accum; sums come
                                # from the ones column)
                                pbp = bh.tile([P, 2, EC, P], F8, tag="pb")
                                nc.scalar.activation(pbp[:], psp[:, :, 0 : EC * P],
                                                     AF.Exp, bias=0.0,
                                                     scale=IWS * IWS if fp8 else 1.0)
                                # causal 0/1 mask on the diagonal blocks
                                # (Pool engine; blk 0 and blk 2, both heads)
                                nc.gpsimd.tensor_tensor(
                                    pbp[:, :, 0, :], pbp[:, :, 0, :],
                                    mask_sb[:].unsqueeze(1).broadcast_to([P, 2, P]),
                                    OP.mult)
                                nc.gpsimd.tensor_tensor(
                                    pbp[:, :, 2, :], pbp[:, :, 2, :],
                                    mask_sb[:].unsqueeze(1).broadcast_to([P, 2, P]),
                                    OP.mult)
                                # AV (token-major out) + ones-column sums;
                                # q-tile1 is one DoubleRow matmul over both
                                # key tiles
                                pav = psav.tile([P, 2, 2, D + 1], F32, tag="av")
                                for hh in range(2):
                                    h = 2 * j + hh
                                    nc.tensor.matmul(pav[:, 0, hh, :],
                                                     pbp[:, hh, 0, :],
                                                     v_b[:, h, 0, 0 : D + 1],
                                                     start=True, stop=True)
                                    nc.tensor.matmul(pav[:, 1, hh, :],
                                                     pbp[:, hh, 1:3, :],
                                                     v_b[:, h, :, 0 : D + 1],
                                                     start=True, stop=True,
                                                     perf_mode=DR)
                                # normalize: o = av / sums (per-partition);
                                # pav is qt-major so o_tm[:, qt, :] is the
                                # [q, hh*64+d] slab the transpose wants
                                rr = stat.tile([P, 2, 2], F32, tag="rr")
                                nc.vector.reciprocal(rr[:], pav[:, :, :, D : D + 1])
                                o_tm = tk.tile([P, 2, P], BF16, tag="otm")
                                nc.vector.tensor_tensor(
                                    o_tm[:].rearrange("p qt (hh d) -> p qt hh d", hh=2),
                                    pav[:, :, :, 0:D],
                                    rr[:].unsqueeze(3).broadcast_to([P, 2, 2, D]),
                                    OP.mult)
                                for qt in range(2):
                                    nc.tensor.transpose(
                                        ptl[:, (2 * lb + qt) * P : (2 * lb + qt + 1) * P],
                                        o_tm[:, qt, :], id_bf[:])
                            copy_out(o_fm[:, j, :], ptl[:])

                        # -- attention out-proj + residual --
                        for i, tt in enumerate(tts):
                            pao = psmm.tile([P, GROUP], F32, tag="mm")
                            if fp8:
                                nc.tensor.matmul(pao[:, :E], o_fm[:, 0:2, i * P : (i + 1) * P],
                                                 wo_sb[:, 0:2, :],
                                                 start=True, stop=False, perf_mode=DR)
                                nc.tensor.matmul(pao[:, :E], o_fm[:, 2, i * P : (i + 1) * P],
                                                 wo_sb[:, 2, :],
                                                 start=False, stop=True)
                            else:
                                for kc in range(EC):
                                    nc.tensor.matmul(pao[:, :E], o_fm[:, kc, i * P : (i + 1) * P],
                                                     wo_sb[:, kc, :],
                                                     start=(kc == 0), stop=(kc == EC - 1))
                            if fp8:
                                nc.vector.scalar_tensor_tensor(
                                    x_tm[tt][:], pao[:, :E], IWS, x_tm[tt][:],
                                    op0=OP.mult, op1=OP.add)
                            elif zero_bias:
                                nc.vector.tensor_tensor(x_tm[tt][:], pao[:, :E], x_tm[tt][:], OP.add)
                            else:
                                t1 = tk.tile([P, E], F32, tag="t1")
                                nc.vector.tensor_tensor(t1[:], pao[:, :E], x_tm[tt][:], OP.add)
                                nc.gpsimd.tensor_tensor(x_tm[tt][:], t1[:], btm_sb[:, 1, :], OP.add)

                        # -- LN2 + transpose --
                        h2_fm = grp.tile([P, EC, GROUP], WDT, tag="hfm")
                        mv_g2, rs_g2, mub_g2 = _ln_stats_group(
                            nc, stat, [x_tm[tt][:] for tt in tts])
                        xh2s = []
                        for i, tt in enumerate(tts):
                            xh2 = tk.tile([P, E], BF16, tag="xhat")
                            ln_apply(x_tm[tt][:], xh2[:], mv_g2, rs_g2, mub_g2, i)
                            xh2s.append(xh2)
                        for kc in range(EC):
                            ptl = pstr.tile([P, GROUP], BF16, tag="tr")
                            for i in range(TPG):
                                nc.tensor.transpose(
                                    ptl[:, i * P : (i + 1) * P],
                                    xh2s[i][:, kc * P : (kc + 1) * P], id_bf[:])
                            copy_out(h2_fm[:, kc, :], ptl[:])

                        # -- FFN: W1 + relu (feature-major hidden) --
                        hf = grp1.tile([P, FC, GROUP], WDT, tag="hf")
                        for m in range(FC):
                            pf = psmm.tile([P, GROUP], F32, tag="mm")
                            if fp8:
                                nc.tensor.matmul(pf[:], w1_sb[:, 0:2, m * P : (m + 1) * P],
                                                 h2_fm[:, 0:2, :],
                                                 start=True, stop=False, perf_mode=DR)
                                nc.tensor.matmul(pf[:], w1_sb[:, 2, m * P : (m + 1) * P],
                                                 h2_fm[:, 2, :],
                                                 start=False, stop=True)
                            else:
                                for kc in range(EC):
                                    nc.tensor.matmul(pf[:], w1_sb[:, kc, m * P : (m + 1) * P],
                                                     h2_fm[:, kc, :],
                                                     start=(kc == 0), stop=(kc == EC - 1))
                            if fp8:
                                if m % 2 == 0:
                                    nc.vector.tensor_scalar(hf[:, m, :], pf[:],
                                                            IWS, 0.0,
                                                            op0=OP.mult, op1=OP.max)
                                else:
                                    nc.scalar.activation(hf[:, m, :], pf[:], AF.Relu,
                                                         bias=0.0, scale=IWS)
                            elif zero_bias:
                                if m % 2 == 0:
                                    nc.vector.tensor_scalar(hf[:, m, :], pf[:],
                                                            0.0, None, op0=OP.max)
                                else:
                                    nc.scalar.activation(hf[:, m, :], pf[:], AF.Relu,
                                                         bias=0.0, scale=1.0)
                            elif m % 2 == 0:
                                nc.vector.tensor_scalar(hf[:, m, :], pf[:],
                                                        c1_sb[:, m : m + 1], 0.0,
                                                        op0=OP.add, op1=OP.max)
                            else:
                                nc.scalar.activation(hf[:, m, :], pf[:], AF.Relu,
                                                     bias=c1_sb[:, m : m + 1], scale=1.0)

                        # -- W2 + residual --
                        for i, tt in enumerate(tts):
                            pw2 = psmm.tile([P, GROUP], F32, tag="mm")
                            if fp8:
                                for c in range(FC // 2):
                                    nc.tensor.matmul(
                                        pw2[:, :E],
                                        hf[:, 2 * c : 2 * c + 2, i * P : (i + 1) * P],
                                        w2_sb[:, 2 * c : 2 * c + 2, :],
                                        start=(c == 0), stop=(c == FC // 2 - 1),
                                        perf_mode=DR)
                            else:
                                for kc in range(FC):
                                    nc.tensor.matmul(pw2[:, :E], hf[:, kc, i * P : (i + 1) * P],
                                                     w2_sb[:, kc, :],
                                                     start=(kc == 0), stop=(kc == FC - 1))
                            if fp8:
                                nc.vector.scalar_tensor_tensor(
                                    x_tm[tt][:], pw2[:, :E], IWS, x_tm[tt][:],
                                    op0=OP.mult, op1=OP.add)
                            elif zero_bias:
                                nc.vector.tensor_tensor(x_tm[tt][:], pw2[:, :E], x_tm[tt][:], OP.add)
                            else:
                                t2 = tk.tile([P, E], F32, tag="t1")
                                nc.vector.tensor_tensor(t2[:], pw2[:, :E], x_tm[tt][:], OP.add)
                                nc.gpsimd.tensor_tensor(x_tm[tt][:], t2[:], btm_sb[:, 2, :], OP.add)

                # ---- final logits ----
                for tt in range(NT):
                    xb = tk.tile([P, E], BF16, tag="xhat")
                    nc.any.tensor_copy(out=xb[:], in_=x_tm[tt][:])
                    xf = tk.tile([P, EC, P], BF16, tag="xf")
                    ptl = pstr.tile([P, GROUP], BF16, tag="tr")
                    for kc in range(EC):
                        nc.tensor.transpose(ptl[:, kc * P : (kc + 1) * P],
                                            xb[:, kc * P : (kc + 1) * P], id_bf[:])
                    copy_out(xf[:], ptl[:, : EC * P])
                    pl = psmm.tile([P, GROUP], F32, tag="mm")
                    for kc in range(EC):
                        nc.tensor.matmul(pl[:, :V], xf[:, kc, :], wl_sb[:, kc, :],
                                         start=(kc == 0), stop=(kc == EC - 1))
                    lg = tk.tile([P, V], F32, tag="lg")
                    if zero_bias:
                        nc.vector.tensor_copy(lg[:], pl[:, :V])
                    else:
                        nc.vector.tensor_tensor(lg[:], pl[:, :V], blr_sb[:], OP.add)
                    nc.sync.dma_start(out[tt * P : (tt + 1) * P, :], lg[:])

    nc.compile()
    return nc


def _prep_host(inputs, fp8=False):
    f32 = np.float32
    bf16 = ml_dtypes.bfloat16
    f8 = ml_dtypes.float8_e4m3

    def wcast(a):
        return (a * WS).astype(f8) if fp8 else a.astype(bf16)

    tokens = np.asarray(inputs["tokens"]).astype(np.int64)
    emb = np.asarray(inputs["emb"], dtype=f32)
    pos_enc = np.asarray(inputs["pos_enc"], dtype=f32)
    Wq = np.asarray(inputs["Wq"], dtype=f32)
    Wk = np.asarray(inputs["Wk"], dtype=f32)
    Wv = np.asarray(inputs["Wv"], dtype=f32)
    Wo = np.asarray(inputs["Wo"], dtype=f32)
    W1 = np.asarray(inputs["W1"], dtype=f32)
    W2 = np.asarray(inputs["W2"], dtype=f32)
    Wl = np.asarray(inputs["Wl"], dtype=f32)
    bq = np.asarray(inputs["bq"], dtype=f32)
    bk = np.asarray(inputs["bk"], dtype=f32)
    bv = np.asarray(inputs["bv"], dtype=f32)
    bo = np.asarray(inputs["bo"], dtype=f32)
    c1 = np.asarray(inputs["c1"], dtype=f32)
    c2 = np.asarray(inputs["c2"], dtype=f32)
    bl = np.asarray(inputs["bl"], dtype=f32)
    g1 = np.asarray(inputs["ln1_g"], dtype=f32)
    b1 = np.asarray(inputs["ln1_b"], dtype=f32)
    g2 = np.asarray(inputs["ln2_g"], dtype=f32)
    b2 = np.asarray(inputs["ln2_b"], dtype=f32)

    scale = D ** -0.5
    wq_f = np.empty((L, E, E), f32)
    wk_f = np.empty((L, E, E), f32)
    wv_f = np.empty((L, E, E), f32)
    w1_f = np.empty((L, E, DFF), f32)
    bq_f = np.empty((L, E), f32)
    bk_f = np.empty((L, E), f32)
    bv_f = np.empty((L, E), f32)
    c1_f = np.empty((L, DFF), f32)
    for l in range(L):
        wq_f[l] = g1[l][:, None] * Wq[l] * scale
        bq_f[l] = (b1[l] @ Wq[l] + bq[l]) * scale
        wk_f[l] = g1[l][:, None] * Wk[l]
        bk_f[l] = b1[l] @ Wk[l] + bk[l]
        wv_f[l] = g1[l][:, None] * Wv[l]
        bv_f[l] = b1[l] @ Wv[l] + bv[l]
        w1_f[l] = g2[l][:, None] * W1[l]
        c1_f[l] = b2[l] @ W1[l] + c1[l]

    common = {
        "embp": np.zeros((P, E), bf16),
        "pose": pos_enc,
        # 0/1 mask, transposed-causal: keep key k for query q iff k <= q
        "maskd": np.triu(np.ones((P, P), f32)).astype(bf16),
        "wq": wcast(wq_f),
        "wk": wcast(wk_f),
        "wv": wcast(wv_f),
        "wo": wcast(Wo),
        "w1": wcast(w1_f),
        "w2": wcast(W2),
        "wl": Wl.astype(bf16),
        "bqf": np.ascontiguousarray(bq_f.reshape(L, EC, P).transpose(0, 2, 1)),
        "bkf": np.ascontiguousarray(bk_f.reshape(L, EC, P).transpose(0, 2, 1)),
        "c1f": np.ascontiguousarray(c1_f.reshape(L, FC, P).transpose(0, 2, 1)),
        "btm": np.ascontiguousarray(
            np.broadcast_to(
                np.stack([bv_f, bo, c2], axis=1)[:, :, None, :], (L, 3, P, E)
            )
        ).astype(f32),
        "blr": np.broadcast_to(bl[None, :], (P, V)).astype(f32),
    }
    common["embp"][:V, :] = emb.astype(bf16)

    in_maps = []
    for c in range(N_CORES):
        tok_c = tokens[c * B_LOC : (c + 1) * B_LOC].reshape(-1)
        oht = np.zeros((P, NTOK), bf16)
        oht[tok_c, np.arange(NTOK)] = 1
        m = dict(common)
        m["oht"] = oht
        in_maps.append(m)
    return in_maps


def _biases_all_zero(inputs):
    zs = [inputs[k] for k in ("bq", "bk", "bv", "bo", "c1", "c2", "bl",
                              "ln1_b", "ln2_b")]
    return all(not np.any(np.asarray(z)) for z in zs)


def kernel(**inputs) -> np.ndarray:
    global _PROG
    zb = _biases_all_zero(inputs)
    if _PROG is None or _PROG[1] != zb:
        _PROG = (build_program(zero_bias=zb), zb)
    nc = _PROG[0]
    in_maps = _prep_host(inputs)
    res = run_bass_kernel_spmd(nc, in_maps, list(range(N_CORES)))
    outs = [res.results[c]["out"].reshape(B_LOC, T, V) for c in range(N_CORES)]
    return np.concatenate(outs, axis=0).astype(np.float32)
